# revision 29
# baseline (speedup 1.0000x reference)
"""Trainium2 Bass kernel for the DagnabbitAutoEncoder DAG scan.

Strategy: level-scheduled DAG (28 levels); mid-size levels are sharded by
node TYPE across the 8 NeuronCores (type t -> cores 2t, 2t+1, replicated MLP
weights) with a per-level AllGather; small levels are REPLICATED on every
core (8x compute, zero collectives).  The whole datapath runs in fp16 with
fp32 PSUM accumulation.

Parent embeddings are fetched with batched transpose-mode dma_gather
(one or two instructions per level instead of one serial indirect-DMA per
128-node column): the gather writes feature-major tiles that feed the MLP
matmuls directly, eliminating all input PE-transposes.  dma_gather indices
are int16, so the embeddings buffer is kept under 64K rows (only nodes that
are actually referenced as parents get a buffer slot) and is addressed
through two overlapping 32K-row windows ("banks"); a gather lane whose slot
lives in the other bank points at a reserved all-zeros row, and the two
banks' tiles are merged with a single vector add.

Per level, per core: gather (1-2 dma_gathers [+ merge add]), 2-layer MLP
(fp16 matmuls, exact-erf Gelu on ACT), PE-transpose back to node-major,
then AllGather of the level block (sharded levels) or a local buffer write
(replicated levels), plus a DMA of the shard block into this core's
shard_out output.  The host merges the 8 partial outputs.
"""

import math
import os

import numpy as np

R = 256
D = 256
NCORES = 8
P = 128
PSUM_N = 448
CH = 448          # gather node-chunk size (2*CH descriptors per instruction)
BANK = 32768
REPL_MAX = 850    # levels with <= this many nodes are replicated on all cores


# ---------------------------------------------------------------------------
# host-side preprocessing
# ---------------------------------------------------------------------------

def _compute_levels(idx):
    n = idx.shape[0]
    depth = np.zeros(R + n, np.int32)
    ia = idx[:, 0]
    ib = idx[:, 1]
    d = depth
    for i in range(n):
        da = d[ia[i]]
        db = d[ib[i]]
        d[R + i] = (da if da > db else db) + 1
    return depth[R:]


def _pad_to(a, size, fill):
    out = np.full(size, fill, np.int64)
    out[: len(a)] = a
    return out


def _plan(idx, types):
    n = idx.shape[0]
    lv = _compute_levels(idx)
    L = int(lv.max()) if n else 0
    ref = np.zeros(R + n, bool)
    ref[idx.ravel()] = True
    order = np.argsort(lv, kind="stable")
    lv_sorted = lv[order]
    level_nodes = []
    lo = 0
    for l in range(1, L + 1):
        hi = lo + np.searchsorted(lv_sorted[lo:], l + 1)
        level_nodes.append(order[lo:hi])
        lo = hi

    modes = ["repl" if len(nodes) <= REPL_MAX else "split"
             for nodes in level_nodes]

    shards = []      # per level: list of 8 node-id arrays (compute order)
    range_list = []  # per level: [(weight_block, c0, c1)] in node-col space
    nref_core = []   # per level (split only): per-core referenced counts
    for l, nodes in enumerate(level_nodes):
        if modes[l] == "repl":
            groups = [nodes[types[nodes] == t] for t in range(4)]
            ordered = np.concatenate(groups)
            ranges = []
            c0 = 0
            for t in range(4):
                c1 = c0 + len(groups[t])
                if c1 > c0:
                    ranges.append((t, c0, c1))
                c0 = c1
            shards.append([ordered] * NCORES)
            range_list.append(ranges)
            nref_core.append(None)
        else:
            per_core = []
            for t in range(4):
                nt = nodes[types[nodes] == t]
                per_core.append(nt[0::2])
                per_core.append(nt[1::2])
            nrefs = []
            for c in range(NCORES):
                s = per_core[c]
                isref = ref[R + s]
                per_core[c] = np.concatenate([s[isref], s[~isref]])
                nrefs.append(int(isref.sum()))
            shards.append(per_core)
            range_list.append(None)
            nref_core.append(nrefs)

    # ---- buffer layout ----------------------------------------------------
    # slot 0 = zeros row, slots 1..R = roots, then per-level blocks.  A zero
    # row is inserted at the bank boundary B1 (start of the first level block
    # that must live in bank1).
    blk_sizes = []
    for l in range(L):
        if modes[l] == "repl":
            blk_sizes.append(len(shards[l][0]))
        else:
            blk_sizes.append(8 * max(nref_core[l]))
    total_wo = 1 + R + sum(blk_sizes)
    cum = 1 + R
    starts = []
    for l in range(L):
        starts.append(cum)
        cum += blk_sizes[l]
    B1 = None
    bl = L
    if total_wo + 1 > BANK:
        for l in range(L):
            if (total_wo + 1) - BANK <= starts[l] <= BANK:
                B1 = starts[l]
                bl = l
        assert B1 is not None, "buffer too large for two banks"
    blks = [starts[l] + (1 if l >= bl else 0) for l in range(L)]
    slots = total_wo + (1 if B1 is not None else 0)
    if B1 is not None:
        assert slots - B1 <= BANK
        assert blks[bl - 1] + blk_sizes[bl - 1] <= BANK if bl > 0 else True

    pos = np.full(R + n, -1, np.int64)
    pos[:R] = 1 + np.arange(R)
    for l in range(L):
        if modes[l] == "repl":
            s = shards[l][0]
            pos[R + s] = blks[l] + np.arange(len(s))
        else:
            S = blk_sizes[l] // 8
            for c in range(NCORES):
                s = shards[l][c]
                nr = nref_core[l][c]
                pos[R + s[:nr]] = blks[l] + c * S + np.arange(nr)

    # ---- gather streams ---------------------------------------------------
    # A dma_gather instruction may carry at most 896 descriptors (the SWDGE
    # descriptor carveout holds <1024).  Each level's nodes are gathered in
    # NODE-range chunks of <=448 nodes: one instruction carries BOTH parents
    # of its node range ([p0 | p1] chunk-local halves), so the MLP of chunk q
    # can run while chunk q+1 is still gathering.
    specs = []
    gcols = 0           # running int16-column offset in the gidx table
    streams = [[] for _ in range(NCORES)]   # per core: list of int16 [16, w]
    src_rows = [[] for _ in range(NCORES)]
    dst_rows = [[] for _ in range(NCORES)]
    soff = 0
    for l in range(L):
        m_sh = [len(s) for s in shards[l]]
        k = max(1, math.ceil(max(m_sh) / P))
        npad = P * k
        chunks = []
        q0 = 0
        while q0 < npad:
            q1 = min(q0 + CH, npad)
            chunks.append((q0, q1))
            q0 = q1

        # per-core parent slots
        mixed = False
        core_streams = []
        for c in range(NCORES):
            s = shards[l][c]
            s0 = pos[idx[s, 0]]
            s1 = pos[idx[s, 1]]
            assert (s0 >= 0).all() and (s1 >= 0).all()
            st = np.stack([_pad_to(s0, npad, 0), _pad_to(s1, npad, 0)])
            core_streams.append(st)          # [2, npad]
            if B1 is not None and int(st.max()) >= B1:
                mixed = True
        boundA = blks[l] if not mixed else B1
        if not mixed:
            assert boundA <= BANK
        # column offsets in the gidx table: per (bank, chunk)
        goffs = {}
        for bank in (["A", "B"] if mixed else ["A"]):
            for qi, (q0, q1) in enumerate(chunks):
                goffs[(bank, qi)] = gcols
                gcols += 2 * (q1 - q0) // 16
        for c in range(NCORES):
            st = core_streams[c]
            for bank in (["A", "B"] if mixed else ["A"]):
                for (q0, q1) in chunks:
                    seg = np.concatenate([st[0, q0:q1], st[1, q0:q1]])
                    if mixed:
                        if bank == "A":
                            seg = np.where(seg < B1, seg, 0)
                        else:
                            seg = np.where(seg >= B1, seg - B1, 0)
                    else:
                        assert int(seg.max()) < boundA
                    streams[c].append(
                        seg.reshape(-1, 16).T.astype(np.int16))

        if modes[l] == "split":
            ranges = [(4, 0, npad)]
            S = blk_sizes[l] // 8
            for c in range(NCORES):
                m = m_sh[c]
                if m:
                    src_rows[c].append(soff * P + np.arange(m))
                    dst_rows[c].append(R + shards[l][c])
        else:
            ranges = range_list[l]
            S = blk_sizes[l]
            m = m_sh[0]
            if m:
                src_rows[0].append(soff * P + np.arange(m))
                dst_rows[0].append(R + shards[l][0])
        specs.append({
            "mode": modes[l], "k": k, "npad": npad,
            "ranges": ranges, "blk": blks[l], "S": S,
            "mixed": mixed, "boundA": boundA,
            "chunks": chunks, "goffs": goffs,
        })
        soff += k

    # idx blocks are replicated into all 8 groups of 16 partitions (one copy
    # per GpSimd Q7 core).
    gidx = np.zeros((NCORES, 128, max(gcols, 1)), np.int16)
    for c in range(NCORES):
        col = 0
        for st in streams[c]:
            w = st.shape[1]
            gidx[c, :, col:col + w] = np.tile(st, (8, 1))
            col += w
        assert col == gcols

    src_rows = [np.concatenate(o) if o else np.zeros(0, np.int64)
                for o in src_rows]
    dst_rows = [np.concatenate(o) if o else np.zeros(0, np.int64)
                for o in dst_rows]
    return {
        "specs": specs, "slots": slots, "B1": B1, "gcols": max(gcols, 1),
        "K": soff, "gidx": gidx,
        "src_rows": src_rows, "dst_rows": dst_rows,
    }


# ---------------------------------------------------------------------------
# Bass program
# ---------------------------------------------------------------------------

def _build_program(specs, slots, B1, gcols, K):
    import concourse.bass as bass
    import concourse.tile as tile
    from concourse import bacc, mybir
    from concourse.masks import make_identity

    F16 = mybir.dt.float16
    F32 = mybir.dt.float32
    I16 = mybir.dt.int16
    AF = mybir.ActivationFunctionType

    nc = bacc.Bacc("TRN2", target_bir_lowering=False, debug=False,
                   num_devices=NCORES)
    t_root = nc.dram_tensor("roots", [R, D], F16, kind="ExternalInput")
    t_w1 = nc.dram_tensor("w1", [P, 5 * 2048], F16, kind="ExternalInput")
    t_w2 = nc.dram_tensor("w2", [P, 5 * 1024], F16, kind="ExternalInput")
    t_b1 = nc.dram_tensor("b1", [P, 5 * 4], F32, kind="ExternalInput")
    t_b2 = nc.dram_tensor("b2", [P, 5 * 2], F32, kind="ExternalInput")
    t_gidx = nc.dram_tensor("gidx", [P, gcols], I16, kind="ExternalInput")
    t_sout = nc.dram_tensor("shard_out", [P * K, D], F16,
                            kind="ExternalOutput")
    buffer = nc.dram_tensor("buffer", [slots, D], F16, kind="Internal",
                            addr_space="Shared")
    groups = [list(range(NCORES))]
    k_max = max(s["k"] for s in specs)
    L = len(specs)

    with tile.TileContext(nc) as tc:
        with (
            tc.tile_pool(name="const", bufs=1) as constp,
            tc.tile_pool(name="sbuf", bufs=2) as sbufp,
            tc.tile_pool(name="psum", bufs=1, space="PSUM") as psump,
            tc.tile_pool(name="dram", bufs=2, space="DRAM") as dramp,
        ):
            ident = constp.tile([P, P], F16)
            make_identity(nc, ident[:])
            w1_sb = constp.tile([P, 5 * 2048], F16)
            nc.sync.dma_start(w1_sb[:], t_w1[:])
            w2_sb = constp.tile([P, 5 * 1024], F16)
            nc.sync.dma_start(w2_sb[:], t_w2[:])
            b1_sb = constp.tile([P, 5 * 4], F32)
            nc.sync.dma_start(b1_sb[:], t_b1[:])
            b2_sb = constp.tile([P, 5 * 2], F32)
            nc.sync.dma_start(b2_sb[:], t_b2[:])
            gidx_sb = constp.tile([P, gcols], I16)
            nc.sync.dma_start(gidx_sb[:], t_gidx[:])
            zt = constp.tile([P, D], F16)
            nc.gpsimd.memset(zt[:], 0.0)

            # buffer init: zero row 0, roots at 1..R, zero row at B1
            nc.sync.dma_start(buffer[0:1, :], zt[0:1, :])
            if B1 is not None:
                nc.sync.dma_start(buffer[B1:B1 + 1, :], zt[0:1, :])
            stg = sbufp.tile([P, (R // P) * D], F16, tag="stg")
            nc.sync.dma_start(
                stg[:], t_root[:].rearrange("(j p) d -> p j d", p=P))
            nc.sync.dma_start(
                buffer[1:R + 1, :].rearrange("(j p) d -> p j d", p=P), stg[:])

            gx_t = {}   # level -> {qi: tile}

            def emit_gather(l, which):
                """Emit dma_gathers for level l, bank `which` ('A'/'B').
                Returns {qi: tile}; tile is [P, 2, 2*(q1-q0)] node-chunk qi
                holding chunk-local [p0 | p1] halves."""
                spec = specs[l]
                tiles = {}
                for qi, (q0, q1) in enumerate(spec["chunks"]):
                    n = 2 * (q1 - q0)
                    tag = f"gx{which}{qi}"
                    t = sbufp.tile([P, 4 * CH], F16, tag=tag,
                                   name=f"{tag}_{l}")[:, : 2 * n]
                    if which == "A":
                        src = buffer[0:spec["boundA"], :]
                    else:
                        src = buffer[B1:slots, :]
                    goff = spec["goffs"][(which, qi)]
                    nc.gpsimd.dma_gather(
                        t.rearrange("p (c n) -> p c n", c=2),
                        src,
                        gidx_sb[:, goff:goff + n // 16],
                        num_idxs=n,
                        num_idxs_reg=n,
                        elem_size=D,
                        transpose=True,
                    )
                    tiles[qi] = t
                return tiles

            def xt_slice(gx, spec, ic, c0, c1):
                h, cc = ic // 2, ic % 2
                for qi, (q0, q1) in enumerate(spec["chunks"]):
                    if q0 <= c0 and c1 <= q1:
                        n = q1 - q0
                        t = gx[qi]
                        base = cc * 2 * n + h * n
                        return t[:, base + (c0 - q0): base + (c1 - q0)]
                raise AssertionError("mm group crosses chunk boundary")

            for l, spec in enumerate(specs):
                k = spec["k"]
                npad = spec["npad"]
                blk = spec["blk"]
                S = spec["S"]

                # ---- gathers ----
                if l == 0:
                    gx_t[0] = emit_gather(0, "A")
                tilesA = gx_t.pop(l)
                if spec["mixed"]:
                    tilesB = emit_gather(l, "B")
                    gx = {}
                    for qi, ta in tilesA.items():
                        gm = sbufp.tile([P, 4 * CH], F16, tag=f"gxM{qi}",
                                        name=f"gxM{qi}_{l}")[:, : ta.shape[1]]
                        nc.vector.tensor_add(gm, ta, tilesB[qi])
                        gx[qi] = gm
                else:
                    gx = tilesA

                # prefetch next level's bank-A gather (independent of this
                # level's pending writes: bank A for mixed levels only holds
                # finalized old blocks).  For non-mixed next levels the
                # gather depends on this level's output, emitted below.
                if l + 1 < L and specs[l + 1]["mixed"]:
                    gx_t[l + 1] = emit_gather(l + 1, "A")

                # ---- MLP ----
                h_sb = [sbufp.tile([P, P * k_max], F16, tag=f"h{oc}",
                                   name=f"h{oc}")[:, : npad] for oc in range(4)]
                et_sb = [sbufp.tile([P, P * k_max], F16, tag=f"et{o2}",
                                    name=f"et{o2}")[:, : npad]
                         for o2 in range(2)]
                bounds = set()
                for g in range(math.ceil(npad / PSUM_N) + 1):
                    bounds.add(min(g * PSUM_N, npad))
                for (q0, q1) in spec["chunks"]:
                    bounds.add(q0)
                    bounds.add(q1)
                bounds = sorted(bounds)
                for g0, g1 in zip(bounds[:-1], bounds[1:]):
                    for wb, r0, r1 in spec["ranges"]:
                        c0 = max(g0, r0)
                        c1 = min(g1, r1)
                        if c1 <= c0:
                            continue
                        ng = c1 - c0
                        cols = slice(c0, c1)
                        for oc in range(4):
                            hp = psump.tile([P, PSUM_N], F32,
                                            tag=f"hp{oc % 2}",
                                            name="hp")[:, :ng]
                            for ic in range(4):
                                w = w1_sb[:, wb * 2048 + ic * 512 + oc * P:
                                          wb * 2048 + ic * 512 + (oc + 1) * P]
                                nc.tensor.matmul(
                                    hp, lhsT=w,
                                    rhs=xt_slice(gx, spec, ic, c0, c1),
                                    start=(ic == 0), stop=(ic == 3))
                            nc.scalar.activation(
                                h_sb[oc][:, cols], hp, AF.Gelu,
                                bias=b1_sb[:, wb * 4 + oc: wb * 4 + oc + 1])
                        for o2 in range(2):
                            ep = psump.tile([P, PSUM_N], F32,
                                            tag=f"ep{o2}", name="ep")[:, :ng]
                            for ic in range(4):
                                w = w2_sb[:, wb * 1024 + ic * 256 + o2 * P:
                                          wb * 1024 + ic * 256 + (o2 + 1) * P]
                                nc.tensor.matmul(
                                    ep, lhsT=w, rhs=h_sb[ic][:, cols],
                                    start=(ic == 0), stop=(ic == 3))
                            nc.vector.tensor_add(
                                et_sb[o2][:, cols], ep,
                                b2_sb[:, wb * 2 + o2: wb * 2 + o2 + 1]
                                .to_broadcast([P, ng]))

                # ---- transpose back to node-major ----
                e_sb = sbufp.tile([P, k_max * D], F16, tag="e",
                                  name="e")[:, : k * D]
                for j in range(k):
                    for o2 in range(2):
                        tp = psump.tile([P, P], F16, tag="tpose", bufs=3,
                                        name="tp")
                        nc.tensor.transpose(
                            tp[:], et_sb[o2][:, j * P:(j + 1) * P], ident[:])
                        nc.vector.tensor_copy(
                            e_sb[:, j * D + o2 * P: j * D + o2 * P + P],
                            tp[:])

                # ---- publish level block ----
                if spec["mode"] == "split":
                    if S > 0:
                        cc_in = dramp.tile([S, D], F16, tag="cc", name="cc")
                        for j in range(math.ceil(S / P)):
                            r0 = j * P
                            r1 = min(r0 + P, S)
                            nc.sync.dma_start(
                                cc_in[r0:r1, :],
                                e_sb[0:r1 - r0, j * D:(j + 1) * D])
                        nc.gpsimd.collective_compute(
                            "AllGather", mybir.AluOpType.bypass,
                            replica_groups=groups,
                            ins=[cc_in[:]],
                            outs=[buffer[blk: blk + NCORES * S, :]])
                else:
                    for j in range(math.ceil(S / P)):
                        r0 = j * P
                        r1 = min(r0 + P, S)
                        nc.sync.dma_start(
                            buffer[blk + r0: blk + r1, :],
                            e_sb[0:r1 - r0, j * D:(j + 1) * D])
                # non-mixed next-level gather depends on this level's block
                if l + 1 < L and not specs[l + 1]["mixed"]:
                    gx_t[l + 1] = emit_gather(l + 1, "A")
                nc.sync.dma_start(
                    t_sout[sum(s["k"] for s in specs[:l]) * P:
                           sum(s["k"] for s in specs[:l]) * P + npad, :]
                    .rearrange("(j p) d -> p j d", p=P), e_sb)
    nc.compile()
    return nc


# ---------------------------------------------------------------------------
# entry point
# ---------------------------------------------------------------------------

_CACHE = {}


def _get_program(key, *args):
    if key not in _CACHE:
        _CACHE[key] = _build_program(*args)
    return _CACHE[key]


def kernel(root_node_embeddings, enc_W1, enc_b1, enc_W2, enc_b2,
           trunk_node_inputs_indices, trunk_node_types):
    from concourse import bass_utils

    root = np.asarray(root_node_embeddings, dtype=np.float32)
    W1 = np.asarray(enc_W1, dtype=np.float32)
    W2 = np.asarray(enc_W2, dtype=np.float32)
    b1 = np.asarray(enc_b1, dtype=np.float32)
    b2 = np.asarray(enc_b2, dtype=np.float32)
    idx = np.asarray(trunk_node_inputs_indices)
    types = np.asarray(trunk_node_types)
    if types.ndim > 1:
        types = types[:, 0]
    types = types.astype(np.int64)
    idx64 = idx.astype(np.int64)
    n = idx64.shape[0]

    plan = _plan(idx64, types)
    specs = plan["specs"]
    key = (tuple((s["k"], s["mode"], s["mixed"], s["S"], tuple(s["ranges"]))
                 for s in specs),
           plan["slots"], plan["B1"])
    nc = _get_program(key, specs, plan["slots"], plan["B1"], plan["gcols"],
                      plan["K"])

    def wtab(W, t):
        blocks = [W[b].reshape(4, P, -1, P).transpose(1, 0, 2, 3).reshape(P, -1)
                  for b in range(4)]
        blocks.append(blocks[t])
        return np.ascontiguousarray(np.concatenate(blocks, 1),
                                    dtype=np.float16)

    def btab(b, t, c):
        blocks = [b[bb].reshape(c, P).T for bb in range(4)]
        blocks.append(blocks[t])
        return np.ascontiguousarray(np.concatenate(blocks, 1),
                                    dtype=np.float32)

    in_maps = []
    for c in range(NCORES):
        t = c // 2
        in_maps.append({
            "roots": np.ascontiguousarray(root, dtype=np.float16),
            "w1": wtab(W1, t),
            "w2": wtab(W2, t),
            "b1": btab(b1, t, 4),
            "b2": btab(b2, t, 2),
            "gidx": np.ascontiguousarray(plan["gidx"][c]),
        })

    res = bass_utils.run_bass_kernel_spmd(
        nc, in_maps, core_ids=list(range(NCORES)),
        trace=bool(int(os.environ.get("DAG_KERNEL_TRACE", "0"))))
    if res.exec_time_ns is not None:
        kernel.last_exec_time_ns = res.exec_time_ns

    out = np.zeros((R + n, D), np.float32)
    out[:R] = root
    for c in range(NCORES):
        dst = plan["dst_rows"][c]
        if len(dst):
            out[dst] = res.results[c]["shard_out"][plan["src_rows"][c]].astype(
                np.float32)
    return out


kernel.last_exec_time_ns = None



# revision 30
# speedup vs baseline: 1.0788x; 1.0788x over previous
"""Trainium2 Bass kernel for the DagnabbitAutoEncoder DAG scan.

Strategy: level-scheduled DAG (28 levels); mid-size levels are sharded by
node TYPE across the 8 NeuronCores (type t -> cores 2t, 2t+1, replicated MLP
weights) with a per-level AllGather; small levels are REPLICATED on every
core (8x compute, zero collectives).  The whole datapath runs in fp16 with
fp32 PSUM accumulation.

Parent embeddings are fetched with batched transpose-mode dma_gather
(one or two instructions per level instead of one serial indirect-DMA per
128-node column): the gather writes feature-major tiles that feed the MLP
matmuls directly, eliminating all input PE-transposes.  dma_gather indices
are int16, so the embeddings buffer is kept under 64K rows (only nodes that
are actually referenced as parents get a buffer slot) and is addressed
through two overlapping 32K-row windows ("banks"); a gather lane whose slot
lives in the other bank points at a reserved all-zeros row, and the two
banks' tiles are merged with a single vector add.

Per level, per core: gather (1-2 dma_gathers [+ merge add]), 2-layer MLP
(fp16 matmuls, exact-erf Gelu on ACT), PE-transpose back to node-major,
then AllGather of the level block (sharded levels) or a local buffer write
(replicated levels), plus a DMA of the shard block into this core's
shard_out output.  The host merges the 8 partial outputs.
"""

import math
import os

import numpy as np

R = 256
D = 256
NCORES = 8
P = 128
PSUM_N = 448
CH = 448          # gather node-chunk size (2*CH descriptors per instruction)
BANK = 32768
REPL_MAX = 400    # levels with <= this many nodes are replicated on all cores


# ---------------------------------------------------------------------------
# host-side preprocessing
# ---------------------------------------------------------------------------

def _compute_levels(idx):
    n = idx.shape[0]
    depth = np.zeros(R + n, np.int32)
    ia = idx[:, 0]
    ib = idx[:, 1]
    d = depth
    for i in range(n):
        da = d[ia[i]]
        db = d[ib[i]]
        d[R + i] = (da if da > db else db) + 1
    return depth[R:]


def _pad_to(a, size, fill):
    out = np.full(size, fill, np.int64)
    out[: len(a)] = a
    return out


def _plan(idx, types):
    n = idx.shape[0]
    lv = _compute_levels(idx)
    L = int(lv.max()) if n else 0
    ref = np.zeros(R + n, bool)
    ref[idx.ravel()] = True
    order = np.argsort(lv, kind="stable")
    lv_sorted = lv[order]
    level_nodes = []
    lo = 0
    for l in range(1, L + 1):
        hi = lo + np.searchsorted(lv_sorted[lo:], l + 1)
        level_nodes.append(order[lo:hi])
        lo = hi

    modes = ["repl" if len(nodes) <= REPL_MAX else "split"
             for nodes in level_nodes]

    shards = []      # per level: list of 8 node-id arrays (compute order)
    range_list = []  # per level: [(weight_block, c0, c1)] in node-col space
    nref_core = []   # per level (split only): per-core referenced counts
    for l, nodes in enumerate(level_nodes):
        if modes[l] == "repl":
            groups = [nodes[types[nodes] == t] for t in range(4)]
            ordered = np.concatenate(groups)
            ranges = []
            c0 = 0
            for t in range(4):
                c1 = c0 + len(groups[t])
                if c1 > c0:
                    ranges.append((t, c0, c1))
                c0 = c1
            shards.append([ordered] * NCORES)
            range_list.append(ranges)
            nref_core.append(None)
        else:
            per_core = []
            for t in range(4):
                nt = nodes[types[nodes] == t]
                per_core.append(nt[0::2])
                per_core.append(nt[1::2])
            nrefs = []
            for c in range(NCORES):
                s = per_core[c]
                isref = ref[R + s]
                per_core[c] = np.concatenate([s[isref], s[~isref]])
                nrefs.append(int(isref.sum()))
            shards.append(per_core)
            range_list.append(None)
            nref_core.append(nrefs)

    # ---- buffer layout ----------------------------------------------------
    # slot 0 = zeros row, slots 1..R = roots, then per-level blocks.  A zero
    # row is inserted at the bank boundary B1 (start of the first level block
    # that must live in bank1).
    blk_sizes = []
    for l in range(L):
        if modes[l] == "repl":
            blk_sizes.append(len(shards[l][0]))
        else:
            blk_sizes.append(8 * max(nref_core[l]))
    total_wo = 1 + R + sum(blk_sizes)
    cum = 1 + R
    starts = []
    for l in range(L):
        starts.append(cum)
        cum += blk_sizes[l]
    B1 = None
    bl = L
    if total_wo + 1 > BANK:
        for l in range(L):
            if (total_wo + 1) - BANK <= starts[l] <= BANK:
                B1 = starts[l]
                bl = l
        assert B1 is not None, "buffer too large for two banks"
    blks = [starts[l] + (1 if l >= bl else 0) for l in range(L)]
    slots = total_wo + (1 if B1 is not None else 0)
    if B1 is not None:
        assert slots - B1 <= BANK
        assert blks[bl - 1] + blk_sizes[bl - 1] <= BANK if bl > 0 else True

    pos = np.full(R + n, -1, np.int64)
    pos[:R] = 1 + np.arange(R)
    for l in range(L):
        if modes[l] == "repl":
            s = shards[l][0]
            pos[R + s] = blks[l] + np.arange(len(s))
        else:
            S = blk_sizes[l] // 8
            for c in range(NCORES):
                s = shards[l][c]
                nr = nref_core[l][c]
                pos[R + s[:nr]] = blks[l] + c * S + np.arange(nr)

    # ---- gather streams ---------------------------------------------------
    # A dma_gather instruction may carry at most 896 descriptors (the SWDGE
    # descriptor carveout holds <1024).  Each level's nodes are gathered in
    # NODE-range chunks of <=448 nodes: one instruction carries BOTH parents
    # of its node range ([p0 | p1] chunk-local halves), so the MLP of chunk q
    # can run while chunk q+1 is still gathering.
    specs = []
    gcols = 0           # running int16-column offset in the gidx table
    streams = [[] for _ in range(NCORES)]   # per core: list of int16 [16, w]
    src_rows = [[] for _ in range(NCORES)]
    dst_rows = [[] for _ in range(NCORES)]
    soff = 0
    for l in range(L):
        m_sh = [len(s) for s in shards[l]]
        k = max(1, math.ceil(max(m_sh) / P))
        npad = P * k
        chunks = []
        q0 = 0
        while q0 < npad:
            q1 = min(q0 + CH, npad)
            chunks.append((q0, q1))
            q0 = q1

        # per-core parent slots
        mixed = False
        core_streams = []
        for c in range(NCORES):
            s = shards[l][c]
            s0 = pos[idx[s, 0]]
            s1 = pos[idx[s, 1]]
            assert (s0 >= 0).all() and (s1 >= 0).all()
            st = np.stack([_pad_to(s0, npad, 0), _pad_to(s1, npad, 0)])
            core_streams.append(st)          # [2, npad]
            if B1 is not None and int(st.max()) >= B1:
                mixed = True
        boundA = blks[l] if not mixed else B1
        if not mixed:
            assert boundA <= BANK
        # column offsets in the gidx table: per (bank, chunk)
        goffs = {}
        for bank in (["A", "B"] if mixed else ["A"]):
            for qi, (q0, q1) in enumerate(chunks):
                goffs[(bank, qi)] = gcols
                gcols += 2 * (q1 - q0) // 16
        for c in range(NCORES):
            st = core_streams[c]
            for bank in (["A", "B"] if mixed else ["A"]):
                for (q0, q1) in chunks:
                    seg = np.concatenate([st[0, q0:q1], st[1, q0:q1]])
                    if mixed:
                        if bank == "A":
                            seg = np.where(seg < B1, seg, 0)
                        else:
                            seg = np.where(seg >= B1, seg - B1, 0)
                    else:
                        assert int(seg.max()) < boundA
                    streams[c].append(
                        seg.reshape(-1, 16).T.astype(np.int16))

        if modes[l] == "split":
            ranges = [(4, 0, npad)]
            S = blk_sizes[l] // 8
            for c in range(NCORES):
                m = m_sh[c]
                if m:
                    src_rows[c].append(soff * P + np.arange(m))
                    dst_rows[c].append(R + shards[l][c])
        else:
            ranges = range_list[l]
            S = blk_sizes[l]
            m = m_sh[0]
            if m:
                src_rows[0].append(soff * P + np.arange(m))
                dst_rows[0].append(R + shards[l][0])
        specs.append({
            "mode": modes[l], "k": k, "npad": npad,
            "ranges": ranges, "blk": blks[l], "S": S,
            "mixed": mixed, "boundA": boundA,
            "chunks": chunks, "goffs": goffs,
        })
        soff += k

    # idx blocks are replicated into all 8 groups of 16 partitions (one copy
    # per GpSimd Q7 core).
    gidx = np.zeros((NCORES, 128, max(gcols, 1)), np.int16)
    for c in range(NCORES):
        col = 0
        for st in streams[c]:
            w = st.shape[1]
            gidx[c, :, col:col + w] = np.tile(st, (8, 1))
            col += w
        assert col == gcols

    src_rows = [np.concatenate(o) if o else np.zeros(0, np.int64)
                for o in src_rows]
    dst_rows = [np.concatenate(o) if o else np.zeros(0, np.int64)
                for o in dst_rows]
    return {
        "specs": specs, "slots": slots, "B1": B1, "gcols": max(gcols, 1),
        "K": soff, "gidx": gidx,
        "src_rows": src_rows, "dst_rows": dst_rows,
    }


# ---------------------------------------------------------------------------
# Bass program
# ---------------------------------------------------------------------------

def _build_program(specs, slots, B1, gcols, K):
    import concourse.bass as bass
    import concourse.tile as tile
    from concourse import bacc, mybir
    from concourse.masks import make_identity

    F16 = mybir.dt.float16
    F32 = mybir.dt.float32
    I16 = mybir.dt.int16
    AF = mybir.ActivationFunctionType

    nc = bacc.Bacc("TRN2", target_bir_lowering=False, debug=False,
                   num_devices=NCORES)
    t_root = nc.dram_tensor("roots", [R, D], F16, kind="ExternalInput")
    t_w1 = nc.dram_tensor("w1", [P, 5 * 2048], F16, kind="ExternalInput")
    t_w2 = nc.dram_tensor("w2", [P, 5 * 1024], F16, kind="ExternalInput")
    t_b1 = nc.dram_tensor("b1", [P, 5 * 4], F32, kind="ExternalInput")
    t_b2 = nc.dram_tensor("b2", [P, 5 * 2], F32, kind="ExternalInput")
    t_gidx = nc.dram_tensor("gidx", [P, gcols], I16, kind="ExternalInput")
    t_sout = nc.dram_tensor("shard_out", [P * K, D], F16,
                            kind="ExternalOutput")
    buffer = nc.dram_tensor("buffer", [slots, D], F16, kind="Internal",
                            addr_space="Shared")
    groups = [list(range(NCORES))]
    k_max = max(s["k"] for s in specs)
    L = len(specs)

    with tile.TileContext(nc) as tc:
        with (
            tc.tile_pool(name="const", bufs=1) as constp,
            tc.tile_pool(name="sbuf", bufs=2) as sbufp,
            tc.tile_pool(name="psum", bufs=1, space="PSUM") as psump,
            tc.tile_pool(name="dram", bufs=2, space="DRAM") as dramp,
        ):
            ident = constp.tile([P, P], F16)
            make_identity(nc, ident[:])
            w1_sb = constp.tile([P, 5 * 2048], F16)
            nc.sync.dma_start(w1_sb[:], t_w1[:])
            w2_sb = constp.tile([P, 5 * 1024], F16)
            nc.sync.dma_start(w2_sb[:], t_w2[:])
            b1_sb = constp.tile([P, 5 * 4], F32)
            nc.sync.dma_start(b1_sb[:], t_b1[:])
            b2_sb = constp.tile([P, 5 * 2], F32)
            nc.sync.dma_start(b2_sb[:], t_b2[:])
            gidx_sb = constp.tile([P, gcols], I16)
            nc.sync.dma_start(gidx_sb[:], t_gidx[:])
            zt = constp.tile([P, D], F16)
            nc.gpsimd.memset(zt[:], 0.0)

            # buffer init: zero row 0, roots at 1..R, zero row at B1
            nc.sync.dma_start(buffer[0:1, :], zt[0:1, :])
            if B1 is not None:
                nc.sync.dma_start(buffer[B1:B1 + 1, :], zt[0:1, :])
            stg = sbufp.tile([P, (R // P) * D], F16, tag="stg")
            nc.sync.dma_start(
                stg[:], t_root[:].rearrange("(j p) d -> p j d", p=P))
            nc.sync.dma_start(
                buffer[1:R + 1, :].rearrange("(j p) d -> p j d", p=P), stg[:])

            gx_t = {}   # level -> {qi: tile}

            def emit_gather(l, which):
                """Emit dma_gathers for level l, bank `which` ('A'/'B').
                Returns {qi: tile}; tile is [P, 2, 2*(q1-q0)] node-chunk qi
                holding chunk-local [p0 | p1] halves."""
                spec = specs[l]
                tiles = {}
                for qi, (q0, q1) in enumerate(spec["chunks"]):
                    n = 2 * (q1 - q0)
                    tag = f"gx{which}{qi}"
                    t = sbufp.tile([P, 4 * CH], F16, tag=tag,
                                   name=f"{tag}_{l}")[:, : 2 * n]
                    if which == "A":
                        src = buffer[0:spec["boundA"], :]
                    else:
                        src = buffer[B1:slots, :]
                    goff = spec["goffs"][(which, qi)]
                    nc.gpsimd.dma_gather(
                        t.rearrange("p (c n) -> p c n", c=2),
                        src,
                        gidx_sb[:, goff:goff + n // 16],
                        num_idxs=n,
                        num_idxs_reg=n,
                        elem_size=D,
                        transpose=True,
                    )
                    tiles[qi] = t
                return tiles

            def xt_slice(gx, spec, ic, c0, c1):
                h, cc = ic // 2, ic % 2
                for qi, (q0, q1) in enumerate(spec["chunks"]):
                    if q0 <= c0 and c1 <= q1:
                        n = q1 - q0
                        t = gx[qi]
                        base = cc * 2 * n + h * n
                        return t[:, base + (c0 - q0): base + (c1 - q0)]
                raise AssertionError("mm group crosses chunk boundary")

            for l, spec in enumerate(specs):
                k = spec["k"]
                npad = spec["npad"]
                blk = spec["blk"]
                S = spec["S"]

                # ---- gathers ----
                if l == 0:
                    gx_t[0] = emit_gather(0, "A")
                tilesA = gx_t.pop(l)
                if spec["mixed"]:
                    tilesB = emit_gather(l, "B")
                    gx = {}
                    for qi, ta in tilesA.items():
                        gm = sbufp.tile([P, 4 * CH], F16, tag=f"gxM{qi}",
                                        name=f"gxM{qi}_{l}")[:, : ta.shape[1]]
                        nc.vector.tensor_add(gm, ta, tilesB[qi])
                        gx[qi] = gm
                else:
                    gx = tilesA

                # prefetch next level's bank-A gather (independent of this
                # level's pending writes: bank A for mixed levels only holds
                # finalized old blocks).  For non-mixed next levels the
                # gather depends on this level's output, emitted below.
                if l + 1 < L and specs[l + 1]["mixed"]:
                    gx_t[l + 1] = emit_gather(l + 1, "A")

                # ---- MLP ----
                h_sb = [sbufp.tile([P, P * k_max], F16, tag=f"h{oc}",
                                   name=f"h{oc}")[:, : npad] for oc in range(4)]
                et_sb = [sbufp.tile([P, P * k_max], F16, tag=f"et{o2}",
                                    name=f"et{o2}")[:, : npad]
                         for o2 in range(2)]
                bounds = set()
                for g in range(math.ceil(npad / PSUM_N) + 1):
                    bounds.add(min(g * PSUM_N, npad))
                for (q0, q1) in spec["chunks"]:
                    bounds.add(q0)
                    bounds.add(q1)
                bounds = sorted(bounds)
                for g0, g1 in zip(bounds[:-1], bounds[1:]):
                    for wb, r0, r1 in spec["ranges"]:
                        c0 = max(g0, r0)
                        c1 = min(g1, r1)
                        if c1 <= c0:
                            continue
                        ng = c1 - c0
                        cols = slice(c0, c1)
                        for oc in range(4):
                            hp = psump.tile([P, PSUM_N], F32,
                                            tag=f"hp{oc % 2}",
                                            name="hp")[:, :ng]
                            for ic in range(4):
                                w = w1_sb[:, wb * 2048 + ic * 512 + oc * P:
                                          wb * 2048 + ic * 512 + (oc + 1) * P]
                                nc.tensor.matmul(
                                    hp, lhsT=w,
                                    rhs=xt_slice(gx, spec, ic, c0, c1),
                                    start=(ic == 0), stop=(ic == 3))
                            nc.scalar.activation(
                                h_sb[oc][:, cols], hp, AF.Gelu,
                                bias=b1_sb[:, wb * 4 + oc: wb * 4 + oc + 1])
                        for o2 in range(2):
                            ep = psump.tile([P, PSUM_N], F32,
                                            tag=f"ep{o2}", name="ep")[:, :ng]
                            for ic in range(4):
                                w = w2_sb[:, wb * 1024 + ic * 256 + o2 * P:
                                          wb * 1024 + ic * 256 + (o2 + 1) * P]
                                nc.tensor.matmul(
                                    ep, lhsT=w, rhs=h_sb[ic][:, cols],
                                    start=(ic == 0), stop=(ic == 3))
                            nc.vector.tensor_add(
                                et_sb[o2][:, cols], ep,
                                b2_sb[:, wb * 2 + o2: wb * 2 + o2 + 1]
                                .to_broadcast([P, ng]))

                # ---- transpose back to node-major ----
                e_sb = sbufp.tile([P, k_max * D], F16, tag="e",
                                  name="e")[:, : k * D]
                for j in range(k):
                    for o2 in range(2):
                        tp = psump.tile([P, P], F16, tag="tpose", bufs=3,
                                        name="tp")
                        nc.tensor.transpose(
                            tp[:], et_sb[o2][:, j * P:(j + 1) * P], ident[:])
                        nc.vector.tensor_copy(
                            e_sb[:, j * D + o2 * P: j * D + o2 * P + P],
                            tp[:])

                # ---- publish level block ----
                if spec["mode"] == "split":
                    if S > 0:
                        cc_in = dramp.tile([S, D], F16, tag="cc", name="cc")
                        for j in range(math.ceil(S / P)):
                            r0 = j * P
                            r1 = min(r0 + P, S)
                            nc.sync.dma_start(
                                cc_in[r0:r1, :],
                                e_sb[0:r1 - r0, j * D:(j + 1) * D])
                        nc.gpsimd.collective_compute(
                            "AllGather", mybir.AluOpType.bypass,
                            replica_groups=groups,
                            ins=[cc_in[:]],
                            outs=[buffer[blk: blk + NCORES * S, :]])
                else:
                    for j in range(math.ceil(S / P)):
                        r0 = j * P
                        r1 = min(r0 + P, S)
                        nc.sync.dma_start(
                            buffer[blk + r0: blk + r1, :],
                            e_sb[0:r1 - r0, j * D:(j + 1) * D])
                # non-mixed next-level gather depends on this level's block
                if l + 1 < L and not specs[l + 1]["mixed"]:
                    gx_t[l + 1] = emit_gather(l + 1, "A")
                nc.sync.dma_start(
                    t_sout[sum(s["k"] for s in specs[:l]) * P:
                           sum(s["k"] for s in specs[:l]) * P + npad, :]
                    .rearrange("(j p) d -> p j d", p=P), e_sb)
    nc.compile()
    return nc


# ---------------------------------------------------------------------------
# entry point
# ---------------------------------------------------------------------------

_CACHE = {}


def _get_program(key, *args):
    if key not in _CACHE:
        _CACHE[key] = _build_program(*args)
    return _CACHE[key]


def kernel(root_node_embeddings, enc_W1, enc_b1, enc_W2, enc_b2,
           trunk_node_inputs_indices, trunk_node_types):
    from concourse import bass_utils

    root = np.asarray(root_node_embeddings, dtype=np.float32)
    W1 = np.asarray(enc_W1, dtype=np.float32)
    W2 = np.asarray(enc_W2, dtype=np.float32)
    b1 = np.asarray(enc_b1, dtype=np.float32)
    b2 = np.asarray(enc_b2, dtype=np.float32)
    idx = np.asarray(trunk_node_inputs_indices)
    types = np.asarray(trunk_node_types)
    if types.ndim > 1:
        types = types[:, 0]
    types = types.astype(np.int64)
    idx64 = idx.astype(np.int64)
    n = idx64.shape[0]

    plan = _plan(idx64, types)
    specs = plan["specs"]
    key = (tuple((s["k"], s["mode"], s["mixed"], s["S"], tuple(s["ranges"]))
                 for s in specs),
           plan["slots"], plan["B1"])
    nc = _get_program(key, specs, plan["slots"], plan["B1"], plan["gcols"],
                      plan["K"])

    def wtab(W, t):
        blocks = [W[b].reshape(4, P, -1, P).transpose(1, 0, 2, 3).reshape(P, -1)
                  for b in range(4)]
        blocks.append(blocks[t])
        return np.ascontiguousarray(np.concatenate(blocks, 1),
                                    dtype=np.float16)

    def btab(b, t, c):
        blocks = [b[bb].reshape(c, P).T for bb in range(4)]
        blocks.append(blocks[t])
        return np.ascontiguousarray(np.concatenate(blocks, 1),
                                    dtype=np.float32)

    in_maps = []
    for c in range(NCORES):
        t = c // 2
        in_maps.append({
            "roots": np.ascontiguousarray(root, dtype=np.float16),
            "w1": wtab(W1, t),
            "w2": wtab(W2, t),
            "b1": btab(b1, t, 4),
            "b2": btab(b2, t, 2),
            "gidx": np.ascontiguousarray(plan["gidx"][c]),
        })

    res = bass_utils.run_bass_kernel_spmd(
        nc, in_maps, core_ids=list(range(NCORES)),
        trace=bool(int(os.environ.get("DAG_KERNEL_TRACE", "0"))))
    if res.exec_time_ns is not None:
        kernel.last_exec_time_ns = res.exec_time_ns

    out = np.zeros((R + n, D), np.float32)
    out[:R] = root
    for c in range(NCORES):
        dst = plan["dst_rows"][c]
        if len(dst):
            out[dst] = res.results[c]["shard_out"][plan["src_rows"][c]].astype(
                np.float32)
    return out


kernel.last_exec_time_ns = None

